# revision 1
# baseline (speedup 1.0000x reference)
"""GQA causal attention (RoPE) for TRN2, 8-core data+tensor parallel.

Sharding: core c in [0,8) handles batch b = c//4 and kv-head group g = c%4
(kv heads {2g, 2g+1}, q heads {4g..4g+3}).  wq/wk/wv column-sharded,
wo row-sharded by head group; host sums the 4 partial wo outputs per batch.

Device layouts (feature-major, "T" = transposed vs reference):
  xT   [DIM, S]      activations, d on partitions
  QT   [128, S]      per q head (head_dim on partitions)
  KT   [128, S]      per kv head
  V    [128k, 256]   natural (position on partitions), 16 k-tiles
  scoresT[k, q]      so softmax denominator is a partition-dim sum (ones matmul)
  attnT [128d, S]    per head -> wo matmul -> outT [DIM, S] (host transposes)

RoPE on [d, s] tiles: out = qt * C + swap_pairs(qt) * S~, with the pair swap
done by a permutation matmul on the PE and C/S~ tables prebuilt on host.

Scheduling (PE is in-order; emission order = execution order):
  - The projection phase is PE-bound while ACT/DVE idle, and the attention
    phase is bound by ACT (exp) / DVE (dac, evictions).  So attention for
    chunks 0 and 1 (and all RoPE) is broken into micro-tasks that are pumped
    between projection PSUM groups of chunks 2/3 — their exp/dac/normalize
    work rides along in the projection window's ACT/DVE slack.
  - Attention chunks 2/3 run after, with scores emitted DEPTH=2 ahead of
    their attnV matmuls, and wo tiles of ready chunks (0,1 then 3) as PE
    filler between heads to cover the exp and normalize chains.
  - Causal mask is applied by accumulating a -1e30 strict-upper block into
    the scores PSUM on the PE itself (no extra engine in the chain).
"""

import json
from collections import deque

import numpy as np
import ml_dtypes

import concourse.bass as bass
import concourse.mybir as mybir
import concourse.tile as tile
import concourse.bass2jax as bass2jax
import concourse.bass_utils as bass_utils
from concourse.bass_utils import run_bass_kernel_spmd


def _split_waits(bir_json: bytes) -> bytes:
    """This walrus build accepts at most ONE sync-wait per instruction (any
    opcode). Tile emits up to ~11. Hoist excess waits onto single-wait Drain
    fillers inserted just before the instruction on the same engine —
    same-engine program order makes this semantically identical."""
    j = json.loads(bir_json)
    changed = False
    for fn in j["functions"]:
        for b in fn["blocks"]:
            out = []
            for ins in b["instructions"]:
                si = ins.get("sync_info")
                ow = si.get("on_wait") if si else None
                if ow and len(ow) > 1:
                    changed = True
                    for k, w in enumerate(ow[:-1]):
                        out.append({
                            "debug": ins.get("debug", 0),
                            "engine": ins["engine"],
                            "ins": [], "outs": [],
                            "name": f"{ins['name']}-w{k}",
                            "opcode": "Drain",
                            "is_reset_sema": False,
                            "sync_info": {"on_update": [], "on_wait": [w]},
                        })
                    si["on_wait"] = [ow[-1]]
                out.append(ins)
            b["instructions"] = out
    return json.dumps(j).encode() if changed else bir_json


_ORIG_COMPILE = bass_utils.compile_bir_kernel


def _patched_compile(bir_json, tmpdir, neff_name="file.neff"):
    return _ORIG_COMPILE(_split_waits(bir_json), tmpdir, neff_name=neff_name)


if getattr(bass2jax.compile_bir_kernel, "__name__", "") != "_patched_compile":
    bass2jax.compile_bir_kernel = _patched_compile
    bass_utils.compile_bir_kernel = _patched_compile

BF16 = mybir.dt.bfloat16
F32 = mybir.dt.float32
Exp = mybir.ActivationFunctionType.Exp
Ln = mybir.ActivationFunctionType.Ln

B, S, DIM = 2, 2048, 2048
N_HEADS, N_KV_HEADS = 16, 8
HEAD_DIM, HALF = 128, 64
N_CORES = 8
QH, KVH = 4, 2            # q / kv heads per core
QW, KW = QH * HEAD_DIM, KVH * HEAD_DIM   # 512, 256
SCALE = 1.0 / float(np.sqrt(HEAD_DIM))

DT = DIM // 128           # 16 contraction tiles for projections
NSC = S // 512            # 4 s-chunks
NKT = S // 128            # 16 k tiles
NET = DIM // 128          # 16 output-feature tiles
DEPTH = 3                 # phase-B: score matmuls emitted ahead of attnV

_BUILT = {}


def _build(nc):
    xt = nc.dram_tensor("xt", [DIM, S], BF16, kind="ExternalInput").ap()
    wq = nc.dram_tensor("wq", [DIM, QW], BF16, kind="ExternalInput").ap()
    wk = nc.dram_tensor("wk", [DIM, KW], BF16, kind="ExternalInput").ap()
    wv = nc.dram_tensor("wv", [DIM, KW], BF16, kind="ExternalInput").ap()
    wo = nc.dram_tensor("wo", [QW, DIM], BF16, kind="ExternalInput").ap()
    cosb = nc.dram_tensor("cosb", [HEAD_DIM, S], BF16, kind="ExternalInput").ap()
    sinb = nc.dram_tensor("sinb", [HEAD_DIM, S], F32, kind="ExternalInput").ap()
    pswp = nc.dram_tensor("pswp", [HEAD_DIM, HEAD_DIM], BF16, kind="ExternalInput").ap()
    ident = nc.dram_tensor("ident", [HEAD_DIM, HEAD_DIM], BF16, kind="ExternalInput").ap()
    maskn = nc.dram_tensor("maskn", [HEAD_DIM, HEAD_DIM], BF16, kind="ExternalInput").ap()
    ones = nc.dram_tensor("ones", [HEAD_DIM, 1], BF16, kind="ExternalInput").ap()
    onerow = nc.dram_tensor("onerow", [1, HEAD_DIM], BF16, kind="ExternalInput").ap()
    outT = nc.dram_tensor("outT", [DIM, S], BF16, kind="ExternalOutput").ap()
    # DRAM bounce buffers for partition-broadcast of per-position reciprocals
    rscr = [nc.dram_tensor(f"rscr{i}", [1, 512], F32).ap() for i in range(NSC * QH)]

    with tile.TileContext(nc) as tc:
        with (
            tc.tile_pool(name="persist", bufs=1) as pp,
            tc.tile_pool(name="trans", bufs=2) as tp,
        ):
            # ---- DMA emission order = queue order. K(c0) runs first on the
            # PE. Startup dispatches spread across idle engine queues:
            # xt(c0)+wq on sync, wk on act, wv/cos/sin on gpsimd — the sync
            # queue's ~600ns/dispatch otherwise caps early HBM throughput.
            xts_all = [[None] * DT for _ in range(NSC)]
            wq_sb, wk_sb, wv_sb = [], [], []
            for d in range(DT):
                xt_t = tp.tile([128, 512], BF16, tag="xts", bufs=36, name=f"xts0_{d}")
                nc.sync.dma_start(xt_t[:], xt[d * 128:(d + 1) * 128, 0:512])
                xts_all[0][d] = xt_t
            for d in range(DT):
                t = pp.tile([128, KW], BF16, tag=f"wk{d}", name=f"wk_sb{d}")
                nc.scalar.dma_start(t[:], wk[d * 128:(d + 1) * 128, :])
                wk_sb.append(t)
            for d in range(DT):
                t = pp.tile([128, KW], BF16, tag=f"wv{d}", name=f"wv_sb{d}")
                nc.gpsimd.dma_start(t[:], wv[d * 128:(d + 1) * 128, :])
                wv_sb.append(t)
            for d in range(DT):
                t = pp.tile([128, QW], BF16, tag=f"wq{d}", name=f"wq_sb{d}")
                nc.sync.dma_start(t[:], wq[d * 128:(d + 1) * 128, :])
                wq_sb.append(t)
            pswp_sb = pp.tile([HEAD_DIM, HEAD_DIM], BF16, tag="pswp", name="pswp_sb")
            nc.gpsimd.dma_start(pswp_sb[:], pswp[:])
            ident_sb = pp.tile([HEAD_DIM, HEAD_DIM], BF16, tag="ident", name="ident_sb")
            nc.gpsimd.dma_start(ident_sb[:], ident[:])
            maskn_sb = pp.tile([HEAD_DIM, HEAD_DIM], BF16, tag="maskn", name="maskn_sb")
            nc.gpsimd.dma_start(maskn_sb[:], maskn[:])
            ones_sb = pp.tile([HEAD_DIM, 1], BF16, tag="ones", name="ones_sb")
            nc.gpsimd.dma_start(ones_sb[:], ones[:])
            onerow_sb = pp.tile([1, HEAD_DIM], BF16, tag="onerow", name="onerow_sb")
            nc.gpsimd.dma_start(onerow_sb[:], onerow[:])
            cos_sb = pp.tile([HEAD_DIM, S], BF16, tag="cos", name="cos_sb")
            nc.gpsimd.dma_start(cos_sb[:], cosb[:])
            sin_sb = pp.tile([HEAD_DIM, S], F32, tag="sin", name="sin_sb")
            nc.gpsimd.dma_start(sin_sb[:], sinb[:])
            for sc in range(1, NSC):
                for d in range(DT):
                    xt_t = tp.tile([128, 512], BF16, tag="xts", bufs=36, name=f"xts{sc}_{d}")
                    nc.sync.dma_start(xt_t[:], xt[d * 128:(d + 1) * 128,
                                                  sc * 512:(sc + 1) * 512])
                    xts_all[sc][d] = xt_t
            wo_sb = []
            for h in range(QH):
                t = pp.tile([128, DIM], BF16, tag=f"wo{h}", name=f"wo_sb{h}")
                nc.sync.dma_start(t[:], wo[h * 128:(h + 1) * 128, :])
                wo_sb.append(t)

            # persistent intermediates
            qtu = [pp.tile([128, S], BF16, tag=f"qtu{h}", name=f"qtu{h}") for h in range(QH)]
            ktu = [pp.tile([128, S], BF16, tag=f"ktu{k}", name=f"ktu{k}") for k in range(KVH)]
            qtr = [pp.tile([128, S], BF16, tag=f"qtr{h}", name=f"qtr{h}") for h in range(QH)]
            ktr = [pp.tile([128, S], BF16, tag=f"ktr{k}", name=f"ktr{k}") for k in range(KVH)]
            v_sb = [pp.tile([128, KW], BF16, tag=f"v{st}", name=f"v{st}") for st in range(NKT)]
            attnT = [pp.tile([128, S], BF16, tag=f"attnT{h}", name=f"attnT{h}") for h in range(QH)]

            def norm_chain(qc, h, den_src, attn_ps):
                """den matmul -> ln -> exp(-x) -> DRAM bounce broadcast ->
                normalize mul producing attnT[h][:, chunk]."""
                qsl = slice(qc * 512, (qc + 1) * 512)
                lnd = tp.tile([1, 512], F32, tag="lnd", bufs=2, name=f"lnd{qc}_{h}")
                nc.scalar.activation(lnd[:], den_src, Ln)
                recip = tp.tile([1, 512], F32, tag="recip", bufs=2, name=f"recip{qc}_{h}")
                nc.scalar.activation(recip[:], lnd[:], Exp, scale=-1.0)
                scr = rscr[qc * QH + h]
                nc.sync.dma_start(scr[:], recip[:])
                rb = tp.tile([128, 512], F32, tag="rb", bufs=2, name=f"rb{qc}_{h}")
                bc = bass.AP(tensor=scr.tensor, offset=scr.offset,
                             ap=[[0, 128]] + list(scr.ap[1:]))
                nc.sync.dma_start(rb[:], bc)
                nc.vector.tensor_mul(attnT[h][:, qsl], attn_ps[:], rb[:])

            def norm_fast2(qc, h, den_src, attn_ps, bc_ps):
                """normalize via PE outer-product broadcast into bc_ps —
                no DRAM round trip, ~2.5us shorter chain."""
                qsl = slice(qc * 512, (qc + 1) * 512)
                lnd = tp.tile([1, 512], F32, tag="lnd", bufs=2, name=f"lnd{qc}_{h}")
                nc.scalar.activation(lnd[:], den_src, Ln)
                recip = tp.tile([1, 512], BF16, tag="frecip", bufs=2, name=f"frecip{qc}_{h}")
                nc.scalar.activation(recip[:], lnd[:], Exp, scale=-1.0)
                nc.tensor.matmul(bc_ps[:], onerow_sb[:], recip[:], start=True, stop=True)
                rbs = tp.tile([128, 512], F32, tag="rb", bufs=2, name=f"rbs{qc}_{h}")
                nc.scalar.copy(rbs[:], bc_ps[:])
                nc.vector.tensor_mul(attnT[h][:, qsl], attn_ps[:], rbs[:])

            # ========== Phase A: projections + interleaved attn(0,1)+rope ==
            with tc.tile_pool(name="pmA", bufs=1, space="PSUM") as pm:
                # psum/partition budget (16KB): qk 3x2K, v 2x1K, shp 1x2K,
                # asc 2x2K (scores for interleaved attn; den rides in row 0
                # of an asc tile), aps 1x2K
                tasks = deque()

                def pump(n):
                    for _ in range(n):
                        if not tasks:
                            return
                        t = tasks.popleft()
                        if next(t, None) is not None:
                            tasks.appendleft(t)

                def rope_one(src, dst, ssl, nm):
                    shp = pm.tile([128, 512], F32, tag="shp", bufs=1, name=f"shp{nm}")
                    nc.tensor.matmul(shp[:], pswp_sb[:], src[:, ssl], start=True, stop=True)
                    t1 = tp.tile([128, 512], BF16, tag="t1", bufs=3, name=f"rt1{nm}")
                    nc.vector.tensor_mul(t1[:], src[:, ssl], cos_sb[:, ssl])
                    t2 = tp.tile([128, 512], BF16, tag="t2", bufs=3, name=f"rt2{nm}")
                    nc.vector.tensor_mul(t2[:], shp[:], sin_sb[:, ssl])
                    nc.vector.tensor_add(dst[:, ssl], t1[:], t2[:])

                def rope_gen(sc):
                    ssl = slice(sc * 512, (sc + 1) * 512)
                    rope_one(qtu[0], qtr[0], ssl, f"q0_{sc}")
                    yield 1
                    for kv in range(KVH):
                        rope_one(ktu[kv], ktr[kv], ssl, f"k{kv}_{sc}")
                        yield 1
                    for h in range(1, QH):
                        rope_one(qtu[h], qtr[h], ssl, f"q{h}_{sc}")
                        yield 1

                def attn_gen(qc, h):
                    """micro-task generator: one kt step (score+exp+dac and
                    the previous step's attnV) per yield."""
                    qsl = slice(qc * 512, (qc + 1) * 512)
                    nkt = 4 * qc + 4
                    kv = h // 2
                    attn_ps = pm.tile([128, 512], F32, tag="aps", bufs=1, name=f"Aattn{qc}_{h}")
                    dac = tp.tile([128, 512], BF16, tag="dac", bufs=2, name=f"Adac{qc}_{h}")
                    pend = []

                    def attn_v(kt, off, span, pt):
                        nc.tensor.matmul(attn_ps[:, off:],
                                         v_sb[kt][:, kv * 128:(kv + 1) * 128],
                                         pt[:, :span], start=(kt == 0),
                                         stop=(kt == nkt - 1))

                    for kt in range(nkt):
                        off = max(0, 128 * kt - 512 * qc)
                        span = 512 - off
                        diag = kt >= 4 * qc
                        scps = pm.tile([128, 512], F32, tag="asc", bufs=2, name=f"Asc{qc}_{h}_{kt}")
                        nc.tensor.matmul(scps[:, :span], ktr[kv][:, kt * 128:(kt + 1) * 128],
                                         qtr[h][:, qc * 512 + off:(qc + 1) * 512],
                                         start=True, stop=not diag)
                        if diag:
                            nc.tensor.matmul(scps[:, :128], ident_sb[:], maskn_sb[:],
                                             start=False, stop=True)
                        pt = tp.tile([128, 512], BF16, tag="pt", bufs=6, name=f"Apt{qc}_{h}_{kt}")
                        nc.scalar.activation(pt[:, :span], scps[:, :span], Exp, scale=SCALE)
                        if kt == 0:
                            nc.vector.tensor_copy(dac[:], pt[:])
                        else:
                            nc.vector.tensor_add(dac[:, off:], dac[:, off:], pt[:, :span])
                        pend.append((kt, off, span, pt))
                        if len(pend) > 1:
                            attn_v(*pend.pop(0))
                        yield 1
                    while pend:
                        attn_v(*pend.pop(0))
                    # den rides in row 0 of an asc-tag psum tile
                    dent = pm.tile([128, 512], F32, tag="asc", bufs=2, name=f"Aden{qc}_{h}")
                    nc.tensor.matmul(dent[0:1, :], ones_sb[:], dac[:], start=True, stop=True)
                    yield 1
                    bc_ps = pm.tile([128, 512], F32, tag="asc", bufs=2, name=f"Abc{qc}_{h}")
                    norm_fast2(qc, h, dent[0:1, :], attn_ps, bc_ps)
                    yield 1

                def proj_q(sc, xts, ssl, after=0):
                    for h in range(QH):
                        ps = pm.tile([128, 512], F32, tag="qk", bufs=2, name=f"qps{sc}_{h}")
                        for d in range(DT):
                            nc.tensor.matmul(ps[:], wq_sb[d][:, h * 128:(h + 1) * 128],
                                             xts[d][:], start=(d == 0), stop=(d == DT - 1))
                        nc.scalar.copy(qtu[h][:, ssl], ps[:])
                        pump(after)

                def proj_k(sc, xts, ssl, after=0):
                    for kv in range(KVH):
                        ps = pm.tile([128, 512], F32, tag="qk", bufs=2, name=f"kps{sc}_{kv}")
                        for d in range(DT):
                            nc.tensor.matmul(ps[:], wk_sb[d][:, kv * 128:(kv + 1) * 128],
                                             xts[d][:], start=(d == 0), stop=(d == DT - 1))
                        nc.scalar.copy(ktu[kv][:, ssl], ps[:])
                        pump(after)

                def proj_v(sc, xts, after=0):
                    for sv in range(4):
                        st = sc * 4 + sv
                        ps = pm.tile([128, KW], F32, tag="v", bufs=2, name=f"vps{st}")
                        for d in range(DT):
                            nc.tensor.matmul(ps[:], xts[d][:, sv * 128:(sv + 1) * 128],
                                             wv_sb[d][:], start=(d == 0), stop=(d == DT - 1))
                        nc.scalar.copy(v_sb[st][:], ps[:])
                        pump(after)

                # c0: K,V,Q (DMA arrival order); no tasks yet
                ssl0 = slice(0, 512)
                proj_k(0, xts_all[0], ssl0)
                proj_v(0, xts_all[0])
                proj_q(0, xts_all[0], ssl0)
                # c1: queue rope(c0)
                tasks.append(rope_gen(0))
                ssl1 = slice(512, 1024)
                proj_q(1, xts_all[1], ssl1, after=1)
                proj_k(1, xts_all[1], ssl1, after=1)
                proj_v(1, xts_all[1], after=1)
                # c2: queue rope(c1) then attn(0)
                tasks.append(rope_gen(1))
                for h in range(QH):
                    tasks.append(attn_gen(0, h))
                ssl2 = slice(1024, 1536)
                proj_q(2, xts_all[2], ssl2, after=4)
                proj_k(2, xts_all[2], ssl2, after=4)
                for h in range(QH):
                    tasks.append(attn_gen(1, h))
                proj_v(2, xts_all[2], after=4)
                # c3: queue rope(c2) then attn(1); rope(c3) units are emitted
                # inline right after the eviction each one depends on, so the
                # DVE reaches them without queueing behind attn(1) norm muls
                # (phase B's first scores need qtr/ktr chunk 3)
                tasks.append(rope_gen(2))
                ssl3 = slice(1536, 2048)
                xts = xts_all[3]
                for h in range(QH):
                    ps = pm.tile([128, 512], F32, tag="qk", bufs=2, name=f"qps3_{h}")
                    for d in range(DT):
                        nc.tensor.matmul(ps[:], wq_sb[d][:, h * 128:(h + 1) * 128],
                                         xts[d][:], start=(d == 0), stop=(d == DT - 1))
                    nc.scalar.copy(qtu[h][:, ssl3], ps[:])
                    rope_one(qtu[h], qtr[h], ssl3, f"q{h}_3")
                    pump(4)
                for kv in range(KVH):
                    ps = pm.tile([128, 512], F32, tag="qk", bufs=2, name=f"kps3_{kv}")
                    for d in range(DT):
                        nc.tensor.matmul(ps[:], wk_sb[d][:, kv * 128:(kv + 1) * 128],
                                         xts[d][:], start=(d == 0), stop=(d == DT - 1))
                    nc.scalar.copy(ktu[kv][:, ssl3], ps[:])
                    rope_one(ktu[kv], ktr[kv], ssl3, f"k{kv}_3")
                    pump(4)
                proj_v(3, xts, after=6)
                while tasks:
                    pump(1)

            # ========== Phase B: attn(3), attn(2) + all wo tiles ==========
            with (
                tc.tile_pool(name="scp", bufs=4, space="PSUM") as scp,
                tc.tile_pool(name="attnp", bufs=2, space="PSUM") as attnp,
                tc.tile_pool(name="wop", bufs=2, space="PSUM") as wop,
            ):
                wo_ctr = [0]

                def wo_tiles(pairs, cast="dve"):
                    for qc2, et in pairs:
                        qsl = slice(qc2 * 512, (qc2 + 1) * 512)
                        wo_ps = wop.tile([128, 512], F32, tag="wo", name=f"wops{qc2}_{et}")
                        for h in range(QH):
                            nc.tensor.matmul(wo_ps[:], wo_sb[h][:, et * 128:(et + 1) * 128],
                                             attnT[h][:, qsl], start=(h == 0), stop=(h == QH - 1))
                        stage = tp.tile([128, 512], BF16, tag="stage", bufs=4, name=f"stage{qc2}_{et}")
                        # eviction engine chosen for per-phase slack: a busy
                        # DVE can head-of-line-block wop bank recycling
                        wo_ctr[0] += 1
                        if cast == "act" or (cast == "alt" and wo_ctr[0] % 2):
                            nc.scalar.copy(stage[:], wo_ps[:])
                        else:
                            nc.vector.tensor_copy(stage[:], wo_ps[:])
                        nc.sync.dma_start(outT[et * 128:(et + 1) * 128, qsl], stage[:])

                def norm_fast(qc, h, den_src, attn_ps):
                    """normalize via PE outer-product broadcast instead of the
                    DRAM bounce — shorter latency for the critical tail."""
                    qsl = slice(qc * 512, (qc + 1) * 512)
                    lnd = tp.tile([1, 512], F32, tag="lnd", bufs=2, name=f"flnd{qc}_{h}")
                    nc.scalar.activation(lnd[:], den_src, Ln)
                    recip = tp.tile([1, 512], BF16, tag="frecip", bufs=2, name=f"frecip{qc}_{h}")
                    nc.scalar.activation(recip[:], lnd[:], Exp, scale=-1.0)
                    bc_ps = wop.tile([128, 512], F32, tag="wo", name=f"bcps{qc}_{h}")
                    nc.tensor.matmul(bc_ps[:], onerow_sb[:], recip[:], start=True, stop=True)
                    rbs = tp.tile([128, 512], F32, tag="rb", bufs=2, name=f"rbs{qc}_{h}")
                    nc.scalar.copy(rbs[:], bc_ps[:])
                    nc.vector.tensor_mul(attnT[h][:, qsl], attn_ps[:], rbs[:])

                def attn_chunk(qc, fph, post_den=(), cast="dve"):
                    # fph: per-head lists of (qc2, et) wo filler tiles;
                    # post_den: fillers emitted between the last head's den
                    # matmul and its normalize consumers.  Each head's
                    # normalize is deferred into the NEXT head's kt loop so
                    # the PE never waits on the recip chain and the DVE mul
                    # never head-of-line-blocks the queue.
                    nkt = 4 * qc + 4
                    pending = [None]

                    def flush_norm():
                        if pending[0] is not None:
                            pending[0]()
                            pending[0] = None

                    for h in range(QH):
                        kv = h // 2
                        attn_ps = attnp.tile([128, 512], F32, tag="attn", name=f"attn{qc}_{h}")
                        dac = tp.tile([128, 512], BF16, tag="dac", bufs=2, name=f"dac{qc}_{h}")

                        def attn_v(kt, off, span, pt):
                            nc.tensor.matmul(attn_ps[:, off:],
                                             v_sb[kt][:, kv * 128:(kv + 1) * 128],
                                             pt[:, :span], start=(kt == 0),
                                             stop=(kt == nkt - 1))

                        # spread wo fillers through the kt loop: keeps wop
                        # bank recycling off the critical path and gives the
                        # exps slack, instead of a burst at head end
                        fillq = deque(fph[h])
                        stride = max(2, (nkt - 4) // max(1, len(fillq)))
                        pend = []
                        for kt in range(nkt):
                            off = max(0, 128 * kt - 512 * qc)
                            span = 512 - off
                            diag = kt >= 4 * qc
                            scps = scp.tile([128, 512], F32, tag="sc", name=f"sc{qc}_{h}_{kt}")
                            nc.tensor.matmul(scps[:, :span], ktr[kv][:, kt * 128:(kt + 1) * 128],
                                             qtr[h][:, qc * 512 + off:(qc + 1) * 512],
                                             start=True, stop=not diag)
                            if diag:
                                nc.tensor.matmul(scps[:, :128], ident_sb[:], maskn_sb[:],
                                                 start=False, stop=True)
                            pt = tp.tile([128, 512], BF16, tag="pt", bufs=6, name=f"pt{qc}_{h}_{kt}")
                            nc.scalar.activation(pt[:, :span], scps[:, :span], Exp, scale=SCALE)
                            if kt == 0:
                                nc.vector.tensor_copy(dac[:], pt[:])
                            else:
                                nc.vector.tensor_add(dac[:, off:], dac[:, off:], pt[:, :span])
                            if kt == 2:
                                flush_norm()
                            pend.append((kt, off, span, pt))
                            if len(pend) > DEPTH:
                                attn_v(*pend.pop(0))
                            if kt >= 4 and fillq and (kt - 4) % stride == 0:
                                wo_tiles([fillq.popleft()], cast=cast)
                        while pend:
                            attn_v(*pend.pop(0))
                        flush_norm()

                        wo_tiles(list(fillq), cast=cast)

                        dent = scp.tile([128, 512], F32, tag="sc", name=f"den{qc}_{h}")
                        nc.tensor.matmul(dent[0:1, :], ones_sb[:], dac[:], start=True, stop=True)
                        if h == QH - 1:
                            wo_tiles(post_den, cast="act")
                            norm_fast(qc, h, dent[0:1, :], attn_ps)
                        else:
                            def mk_norm(h=h, dent=dent, attn_ps=attn_ps):
                                norm_fast(qc, h, dent[0:1, :], attn_ps)
                            pending[0] = mk_norm

                A32 = [(0, et) for et in range(NET)] + [(1, et) for et in range(NET)]
                B16 = [(3, et) for et in range(NET)]
                attn_chunk(3, [[], A32[0:9], A32[9:18], A32[18:26]])
                attn_chunk(2, [A32[26:32], B16[0:5], B16[5:10], B16[10:13]],
                           post_den=B16[13:16], cast="dve")
                wo_tiles([(2, et) for et in range(NET)], cast="act")
    return nc


def get_nc():
    if "nc" not in _BUILT:
        nc = bass.Bass("TRN2", debug=False, enable_asserts=False,
                       num_devices=N_CORES)
        _BUILT["nc"] = _build(nc)
    return _BUILT["nc"]


def prepare_in_maps(x, pos_cos, pos_sin, wq, wk, wv, wo):
    bf = ml_dtypes.bfloat16
    x = np.asarray(x, np.float32)
    pos_cos = np.asarray(pos_cos, np.float32)
    pos_sin = np.asarray(pos_sin, np.float32)
    wq = np.asarray(wq, np.float32)
    wk = np.asarray(wk, np.float32)
    wv = np.asarray(wv, np.float32)
    wo = np.asarray(wo, np.float32)

    pair = np.repeat(np.arange(HALF), 2)          # d -> d//2
    C = pos_cos.T[pair]                           # [128, S]
    Sm = pos_sin.T[pair].copy()                   # [128, S]
    Sm[0::2] *= -1.0                              # even d: -sin, odd d: +sin
    pswap = np.zeros((128, 128), np.float32)
    pswap[np.arange(128), np.arange(128) ^ 1] = 1.0
    identm = np.eye(128, dtype=np.float32)
    # maskneg[k, q] = 0 where q >= k (keep), -1e30 where q < k (mask)
    maskneg = np.where(np.triu(np.ones((128, 128), np.float32)) > 0, 0.0, -1e30)
    ones = np.ones((128, 1), np.float32)

    common = {
        "cosb": C.astype(bf), "sinb": Sm.astype(np.float32),
        "pswp": pswap.astype(bf), "ident": identm.astype(bf),
        "maskn": maskneg.astype(bf), "ones": ones.astype(bf),
        "onerow": np.ones((1, 128), np.float32).astype(bf),
    }
    in_maps = []
    for c in range(N_CORES):
        b, g = divmod(c, 4)
        in_maps.append(dict(
            xt=np.ascontiguousarray(x[b].T).astype(bf),
            wq=wq[:, QW * g:QW * (g + 1)].astype(bf),
            wk=wk[:, KW * g:KW * (g + 1)].astype(bf),
            wv=wv[:, KW * g:KW * (g + 1)].astype(bf),
            wo=wo[QW * g:QW * (g + 1), :].astype(bf),
            **common,
        ))
    return in_maps


def gather(results):
    out = np.zeros((B, S, DIM), np.float32)
    for c in range(N_CORES):
        b = c // 4
        out[b] += results[c]["outT"].T.astype(np.float32)
    return out


def run(inputs, trace=False, tmpdir=None):
    nc = get_nc()
    in_maps = prepare_in_maps(**inputs)
    res = run_bass_kernel_spmd(nc, in_maps, list(range(N_CORES)),
                               trace=trace, tmpdir=tmpdir)
    return gather(res.results), res


def kernel(x, pos_cos, pos_sin, wq, wk, wv, wo):
    out, _ = run(dict(x=x, pos_cos=pos_cos, pos_sin=pos_sin,
                      wq=wq, wk=wk, wv=wv, wo=wo))
    return out

